# revision 12
# baseline (speedup 1.0000x reference)
"""MoE layer (16 experts, top-2) on 8 Trainium2 NeuronCores, expert-parallel.

Contract: kernel(**inputs) takes the full unsharded inputs
(x [4,2048,1024], gate_w [1024,16], Wg/Wu [16,1024,2816], Wd [16,2816,1024])
and returns (out [4,2048,1024] float32, aux_loss float32 scalar), matching
reference._moe_forward.

Strategy:
  - Router (x @ gate_w, softmax, top-2, combine weights, aux loss) runs on
    host via JAX-CPU (0.05% of FLOPs), mirroring the reference numerics.
  - Tokens are dispatched per expert on host; each of the 8 cores processes
    2 experts (expert-parallel, per the sharding hint). Per expert the core
    runs the FFN: H^T = [silu(x@Wg) * (x@Wu)]^T computed directly in
    transposed layout (weights stationary on the PE, token activations
    streaming), then Y = H @ Wd accumulated over I-tiles in PSUM with H^T
    tiles as the stationary operand. Combine weights are applied on-device
    as a per-partition scale during the PSUM->SBUF copy.
  - Host scatters per-expert outputs back (indices within one expert are
    unique, so a fancy-index add is exact).

Matmul operands are bf16 (PSUM accumulation and everything after stays
fp32): fp32 matmul on TRN2 runs as a two-pass hi/lo split with serialized
weight loads (~4x slower end-to-end, measured 2.11ms vs 0.54ms), while bf16
streams one column/cycle with fast weight loads hidden behind the stream.
Measured output error vs the fp32 reference: ~4e-3 of max|out|.

Measured on 8 trn2 cores: HW exec ~539us/core vs a 507us pure PE-stream
floor (tail barrier ~15us + preamble ~7us + HAM ramp account for the rest).
"""

import sys
import numpy as np
import ml_dtypes

BF16 = ml_dtypes.bfloat16

if '/opt/trn_rl_repo' not in sys.path:
    sys.path.insert(0, '/opt/trn_rl_repo')

NUM_EXPERTS = 16
TOP_K = 2
AUX_LOSS_COEF = 0.01
D = 1024
I = 2816
KT = D // 128        # 8 contraction tiles for MM1
MT = I // 128        # 22 I tiles
TB = 384             # token block (PSUM free dim for MM1, 3 blocks of 128)
N_CORES = 8
E_PER_CORE = NUM_EXPERTS // N_CORES

_BUILD_CACHE = {}


def _build(C):
    """Emit the Bass/Tile program for capacity C tokens per expert."""
    import concourse.mybir as mybir
    import concourse.tile as tile
    from concourse import bacc

    f32 = mybir.dt.float32
    bf16 = mybir.dt.bfloat16
    Act = mybir.ActivationFunctionType
    NTG = C // TB            # token groups (of TB tokens)
    NMT = TB // 128          # 128-token tiles per group
    NT = C // 128            # total 128-token tiles
    CW = min(512, D)         # output column chunk for MM2
    NCH = D // CW

    nc = bacc.Bacc("TRN2", target_bir_lowering=False, debug=False,
                   num_devices=N_CORES)

    xT = nc.dram_tensor("xT", [E_PER_CORE, KT, 128, C], bf16,
                        kind="ExternalInput").ap()
    Wg = nc.dram_tensor("Wg", [E_PER_CORE, MT, 128, KT, 128], bf16,
                        kind="ExternalInput").ap()
    Wu = nc.dram_tensor("Wu", [E_PER_CORE, MT, 128, KT, 128], bf16,
                        kind="ExternalInput").ap()
    Wd = nc.dram_tensor("Wd", [E_PER_CORE, MT, 128, D], bf16,
                        kind="ExternalInput").ap()
    wv = nc.dram_tensor("wv", [E_PER_CORE, NT, 128], f32,
                        kind="ExternalInput").ap()
    Y = nc.dram_tensor("Y", [E_PER_CORE, C, D], f32,
                       kind="ExternalOutput").ap()

    with tile.TileContext(nc) as tc:
        with tc.tile_pool(name="sb_ht", bufs=1) as sb_ht, \
             tc.tile_pool(name="sb_xt", bufs=1) as sb_xt, \
             tc.tile_pool(name="sb_w", bufs=2) as sb_w, \
             tc.tile_pool(name="sb_wd", bufs=1) as sb_wd, \
             tc.tile_pool(name="sb_act", bufs=3) as sb_act, \
             tc.tile_pool(name="sb_out", bufs=4) as sb_out, \
             tc.tile_pool(name="sb_misc", bufs=1) as sb_misc, \
             tc.tile_pool(name="ps", bufs=1, space="PSUM") as psum:
            # Pre-warm the PE HAM clock gate during the DMA preamble: junk
            # matmuls with no data deps issue immediately and keep the PE
            # busy past the 3.4us activity window, so real matmuls start
            # at 2.4GHz instead of 1.2.
            warm_sb = sb_misc.tile([128, 256], bf16, tag="warm", name="warm_sb")
            nc.vector.memset(warm_sb, 0.0)
            warm_ps = psum.tile([128, 128], f32, tag="warm", name="warm_ps")
            for i in range(40):
                nc.tensor.matmul(warm_ps, warm_sb[:, :128],
                                 warm_sb[:, 128:], start=(i == 0),
                                 stop=(i == 39))
            def emit_preamble(e):
                # First m-tile's weights go first so MM1(m=0) can start
                # as soon as xt[0] lands.
                wg0 = sb_w.tile([128, KT, 128], bf16, tag="wg", name="wg_t")
                nc.sync.dma_start(out=wg0, in_=Wg[e, 0])
                wu0 = sb_w.tile([128, KT, 128], bf16, tag="wu", name="wu_t")
                nc.sync.dma_start(out=wu0, in_=Wu[e, 0])
                # Token activations, transposed: 8 k-tiles of [128, C].
                xt = []
                for k in range(KT):
                    xt_t = sb_xt.tile([128, C], bf16, tag=f"xt{k}", name=f"xt{k}")
                    nc.sync.dma_start(out=xt_t, in_=xT[e, k])
                    xt.append(xt_t)
                wv_sb = sb_misc.tile([128, NT], f32, tag="wv", name="wv_sb")
                nc.sync.dma_start(out=wv_sb, in_=wv[e].rearrange("m p -> p m"))
                return wg0, wu0, xt, wv_sb

            pre = emit_preamble(0)
            for e in range(E_PER_CORE):
                wg0, wu0, xt, wv_sb = pre

                # H^T, all I tiles x all tokens, fp32 resident in SBUF.
                ht = sb_ht.tile([128, MT * C], bf16, tag="ht", name="ht", bufs=2)

                # ---- Phase A: H^T[m] = silu(Wg_m^T x^T) * (Wu_m^T x^T) ----
                wd_tiles = []
                if True:
                    for m in range(MT):
                        if m == 0:
                            wg_t, wu_t = wg0, wu0
                        else:
                            wg_t = sb_w.tile([128, KT, 128], bf16, tag="wg",
                                             name="wg_t")
                            nc.sync.dma_start(out=wg_t, in_=Wg[e, m])
                            wu_t = sb_w.tile([128, KT, 128], bf16, tag="wu",
                                             name="wu_t")
                            nc.sync.dma_start(out=wu_t, in_=Wu[e, m])
                        # Preload Wd[m] now; phase B consumes it from SBUF.
                        wd_t = sb_wd.tile([128, D], bf16, tag=f"wd{m}",
                                          name=f"wd{m}")
                        nc.sync.dma_start(out=wd_t, in_=Wd[e, m])
                        wd_tiles.append(wd_t)
                        hg = [psum.tile([128, TB], f32, tag=f"p{t}", name=f"hg{t}")
                              for t in range(NTG)]
                        hu = [psum.tile([128, TB], f32, tag=f"q{t}", name=f"hu{t}")
                              for t in range(NTG)]
                        for wt, ps in ((wg_t, hg), (wu_t, hu)):
                            for k in range(KT):
                                lhsT = wt[:, k, :]
                                for t in range(NTG):
                                    nc.tensor.matmul(
                                        ps[t], lhsT,
                                        xt[k][:, t * TB:(t + 1) * TB],
                                        start=(k == 0), stop=(k == KT - 1))
                        for t in range(NTG):
                            sl = sb_act.tile([128, TB], f32, tag="silu", name="sl")
                            nc.scalar.activation(sl, hg[t], Act.Silu)
                            nc.vector.tensor_mul(
                                ht[:, m * C + t * TB: m * C + (t + 1) * TB],
                                sl, hu[t])

                # Next expert's preamble DMAs go ahead of phase B's
                # Y-store DMAs in the sync queue; their SBUF slots are
                # already free once phase A stops reading them.
                if e + 1 < E_PER_CORE:
                    pre = emit_preamble(e + 1)

                # ---- Phase B: Y[tg] = sum_m H^T[m, tg]^T @ Wd[m] ----
                if True:
                    for tg in range(NTG):
                        tagn = ["p", "q"]
                        ys = [[psum.tile([128, CW], f32,
                                       tag=f"{tagn[n % len(tagn)]}{mt}",
                                       name=f"ys{mt}_{n}")
                               for n in range(NCH)] for mt in range(NMT)]
                        for m in range(MT):
                            wd_t = wd_tiles[m]
                            for mt in range(NMT):
                                col = m * C + tg * TB + mt * 128
                                lhsT = ht[:, col:col + 128]
                                for n in range(NCH):
                                    nc.tensor.matmul(
                                        ys[mt][n], lhsT,
                                        wd_t[:, n * CW:(n + 1) * CW],
                                        start=(m == 0), stop=(m == MT - 1))
                        for mt in range(NMT):
                            scol = tg * NMT + mt
                            for n in range(NCH):
                                yo = sb_out.tile([128, CW], f32, tag="yo",
                                                 name="yo")
                                nc.scalar.activation(
                                    yo, ys[mt][n], Act.Copy,
                                    scale=wv_sb[:, scol:scol + 1])
                                row = tg * TB + mt * 128
                                nc.sync.dma_start(
                                    out=Y[e, row:row + 128,
                                          n * CW:(n + 1) * CW],
                                    in_=yo)
    nc.compile()
    return nc


def _get_nc(C):
    if C not in _BUILD_CACHE:
        _BUILD_CACHE[C] = _build(C)
    return _BUILD_CACHE[C]


def _router_host(x_flat, gate_w):
    """Replicate the reference router + aux loss on JAX-CPU."""
    import jax
    import jax.numpy as jnp

    cpu = jax.devices("cpu")[0]
    with jax.default_device(cpu):
        xj = jnp.asarray(x_flat)
        gj = jnp.asarray(gate_w)
        logits = xj @ gj
        probs = jax.nn.softmax(logits, axis=-1)
        topk_w, topk_idx = jax.lax.top_k(probs, TOP_K)
        topk_wn = topk_w / jnp.sum(topk_w, axis=-1, keepdims=True)
        # aux loss: tokens_per_expert . router_prob_per_expert
        onehot = jax.nn.one_hot(topk_idx, NUM_EXPERTS, dtype=xj.dtype)
        expert_mask = onehot.sum(axis=1)              # [T, E] (K summed)
        tokens_per_expert = expert_mask.mean(axis=0)
        router_prob_per_expert = probs.mean(axis=0)
        aux = AUX_LOSS_COEF * jnp.sum(
            tokens_per_expert * router_prob_per_expert) * NUM_EXPERTS
    return (np.asarray(topk_wn), np.asarray(topk_idx),
            np.asarray(aux, dtype=np.float32))


def kernel(x, gate_w, Wg, Wu, Wd):
    from concourse.bass_utils import run_bass_kernel_spmd

    x = np.asarray(x, dtype=np.float32)
    gate_w = np.asarray(gate_w, dtype=np.float32)
    Wg = np.asarray(Wg, dtype=np.float32)
    Wu = np.asarray(Wu, dtype=np.float32)
    Wd = np.asarray(Wd, dtype=np.float32)

    B, S, _ = x.shape
    T = B * S
    x_flat = x.reshape(T, D)

    topk_wn, topk_idx, aux = _router_host(x_flat, gate_w)

    # Group (token, weight) pairs by expert.
    flat_e = topk_idx.reshape(-1).astype(np.int64)
    flat_w = topk_wn.reshape(-1).astype(np.float32)
    flat_t = np.repeat(np.arange(T, dtype=np.int64), TOP_K)
    order = np.argsort(flat_e, kind="stable")
    sorted_e = flat_e[order]
    tok_by_e = flat_t[order]
    w_by_e = flat_w[order]
    starts = np.searchsorted(sorted_e, np.arange(NUM_EXPERTS + 1))
    counts = np.diff(starts)

    C = max(TB * 3, int(-(-counts.max() // TB)) * TB)
    nc = _get_nc(C)

    x_bf = x_flat.astype(BF16)
    Wg_bf = Wg.astype(BF16)
    Wu_bf = Wu.astype(BF16)
    Wd_bf = Wd.astype(BF16)

    in_maps = []
    tok_lists = []
    for c in range(N_CORES):
        xT_c = np.zeros((E_PER_CORE, KT, 128, C), dtype=BF16)
        wv_c = np.zeros((E_PER_CORE, C // 128, 128), dtype=np.float32)
        Wg_c = np.empty((E_PER_CORE, MT, 128, KT, 128), dtype=BF16)
        Wu_c = np.empty_like(Wg_c)
        Wd_c = np.empty((E_PER_CORE, MT, 128, D), dtype=BF16)
        toks = []
        for j in range(E_PER_CORE):
            e = c * E_PER_CORE + j
            idx = tok_by_e[starts[e]:starts[e + 1]]
            w = w_by_e[starts[e]:starts[e + 1]]
            cnt = len(idx)
            toks.append(idx)
            # [D, cnt] -> [KT, 128, cnt]
            xg = x_bf[idx].T.reshape(KT, 128, cnt)
            xT_c[j, :, :, :cnt] = xg
            wv_c[j].reshape(-1)[:cnt] = w
            # Wg[e]: [D, I] -> [m, p(D in k-tile), k, c(I col)]
            Wg_c[j] = Wg_bf[e].reshape(KT, 128, MT, 128).transpose(2, 1, 0, 3)
            Wu_c[j] = Wu_bf[e].reshape(KT, 128, MT, 128).transpose(2, 1, 0, 3)
            Wd_c[j] = Wd_bf[e].reshape(MT, 128, D)
        tok_lists.append(toks)
        in_maps.append({"xT": xT_c, "Wg": Wg_c, "Wu": Wu_c, "Wd": Wd_c,
                        "wv": wv_c})

    res = run_bass_kernel_spmd(nc, in_maps, core_ids=list(range(N_CORES)))

    out_flat = np.zeros((T, D), dtype=np.float32)
    for c in range(N_CORES):
        Yc = res.results[c]["Y"]
        for j in range(E_PER_CORE):
            idx = tok_lists[c][j]
            out_flat[idx] += Yc[j, :len(idx)]
    return out_flat.reshape(B, S, D), aux


# revision 13
# speedup vs baseline: 1.0090x; 1.0090x over previous
"""MoE layer (16 experts, top-2) on 8 Trainium2 NeuronCores, expert-parallel.

Contract: kernel(**inputs) takes the full unsharded inputs
(x [4,2048,1024], gate_w [1024,16], Wg/Wu [16,1024,2816], Wd [16,2816,1024])
and returns (out [4,2048,1024] float32, aux_loss float32 scalar), matching
reference._moe_forward.

Strategy:
  - Router (x @ gate_w, softmax, top-2, combine weights, aux loss) runs on
    host via JAX-CPU (0.05% of FLOPs), mirroring the reference numerics.
  - Tokens are dispatched per expert on host; each of the 8 cores processes
    2 experts (expert-parallel, per the sharding hint). Per expert the core
    runs the FFN: H^T = [silu(x@Wg) * (x@Wu)]^T computed directly in
    transposed layout (weights stationary on the PE, token activations
    streaming), then Y = H @ Wd accumulated over I-tiles in PSUM with H^T
    tiles as the stationary operand. Combine weights are applied on-device
    as a per-partition scale during the PSUM->SBUF copy.
  - Host scatters per-expert outputs back (indices within one expert are
    unique, so a fancy-index add is exact).

Matmul operands are bf16 (PSUM accumulation and everything after stays
fp32): fp32 matmul on TRN2 runs as a two-pass hi/lo split with serialized
weight loads (~4x slower end-to-end, measured 2.11ms vs 0.54ms), while bf16
streams one column/cycle with fast weight loads hidden behind the stream.
Measured output error vs the fp32 reference: ~4e-3 of max|out|.

Measured on 8 trn2 cores: HW exec ~539us/core vs a 507us pure PE-stream
floor (tail barrier ~15us + preamble ~7us + HAM ramp account for the rest).
"""

import sys
import numpy as np
import ml_dtypes

BF16 = ml_dtypes.bfloat16

if '/opt/trn_rl_repo' not in sys.path:
    sys.path.insert(0, '/opt/trn_rl_repo')

NUM_EXPERTS = 16
TOP_K = 2
AUX_LOSS_COEF = 0.01
D = 1024
I = 2816
KT = D // 128        # 8 contraction tiles for MM1
MT = I // 128        # 22 I tiles
TB = 384             # token block (PSUM free dim for MM1, 3 blocks of 128)
N_CORES = 8
E_PER_CORE = NUM_EXPERTS // N_CORES

_BUILD_CACHE = {}


def _build(C):
    """Emit the Bass/Tile program for capacity C tokens per expert."""
    import concourse.mybir as mybir
    import concourse.tile as tile
    from concourse import bacc

    f32 = mybir.dt.float32
    bf16 = mybir.dt.bfloat16
    Act = mybir.ActivationFunctionType
    NTG = C // TB            # token groups (of TB tokens)
    NMT = TB // 128          # 128-token tiles per group
    NT = C // 128            # total 128-token tiles
    CW = min(512, D)         # output column chunk for MM2
    NCH = D // CW

    nc = bacc.Bacc("TRN2", target_bir_lowering=False, debug=False,
                   num_devices=N_CORES)

    xT = nc.dram_tensor("xT", [E_PER_CORE, KT, 128, C], bf16,
                        kind="ExternalInput").ap()
    Wg = nc.dram_tensor("Wg", [E_PER_CORE, MT, 128, KT, 128], bf16,
                        kind="ExternalInput").ap()
    Wu = nc.dram_tensor("Wu", [E_PER_CORE, MT, 128, KT, 128], bf16,
                        kind="ExternalInput").ap()
    Wd = nc.dram_tensor("Wd", [E_PER_CORE, MT, 128, D], bf16,
                        kind="ExternalInput").ap()
    wv = nc.dram_tensor("wv", [E_PER_CORE, NT, 128], f32,
                        kind="ExternalInput").ap()
    Y = nc.dram_tensor("Y", [E_PER_CORE, C, D], f32,
                       kind="ExternalOutput").ap()

    with tile.TileContext(nc) as tc:
        with tc.tile_pool(name="sb_ht", bufs=1) as sb_ht, \
             tc.tile_pool(name="sb_xt", bufs=1) as sb_xt, \
             tc.tile_pool(name="sb_w", bufs=2) as sb_w, \
             tc.tile_pool(name="sb_wd", bufs=1) as sb_wd, \
             tc.tile_pool(name="sb_act", bufs=3) as sb_act, \
             tc.tile_pool(name="sb_out", bufs=6) as sb_out, \
             tc.tile_pool(name="sb_misc", bufs=1) as sb_misc, \
             tc.tile_pool(name="ps", bufs=1, space="PSUM") as psum:
            # Pre-warm the PE HAM clock gate during the DMA preamble: junk
            # matmuls with no data deps issue immediately and keep the PE
            # busy past the 3.4us activity window, so real matmuls start
            # at 2.4GHz instead of 1.2.
            warm_sb = sb_misc.tile([128, 256], bf16, tag="warm", name="warm_sb")
            nc.vector.memset(warm_sb, 0.0)
            warm_ps = psum.tile([128, 128], f32, tag="warm", name="warm_ps")
            for i in range(52):
                nc.tensor.matmul(warm_ps, warm_sb[:, :128],
                                 warm_sb[:, 128:], start=(i == 0),
                                 stop=(i == 51))
            def emit_preamble(e):
                # First m-tile's weights go first so MM1(m=0) can start
                # as soon as xt[0] lands.
                wg0 = sb_w.tile([128, KT, 128], bf16, tag="wg", name="wg_t")
                nc.sync.dma_start(out=wg0, in_=Wg[e, 0])
                wu0 = sb_w.tile([128, KT, 128], bf16, tag="wu", name="wu_t")
                nc.sync.dma_start(out=wu0, in_=Wu[e, 0])
                # Token activations, transposed: 8 k-tiles of [128, C].
                xt = []
                for k in range(KT):
                    xt_t = sb_xt.tile([128, C], bf16, tag=f"xt{k}", name=f"xt{k}")
                    nc.sync.dma_start(out=xt_t, in_=xT[e, k])
                    xt.append(xt_t)
                wv_sb = sb_misc.tile([128, NT], f32, tag="wv", name="wv_sb")
                nc.sync.dma_start(out=wv_sb, in_=wv[e].rearrange("m p -> p m"))
                return wg0, wu0, xt, wv_sb

            pre = emit_preamble(0)
            for e in range(E_PER_CORE):
                wg0, wu0, xt, wv_sb = pre

                # H^T, all I tiles x all tokens, fp32 resident in SBUF.
                ht = sb_ht.tile([128, MT * C], bf16, tag="ht", name="ht", bufs=2)

                # ---- Phase A: H^T[m] = silu(Wg_m^T x^T) * (Wu_m^T x^T) ----
                wd_tiles = []
                if True:
                    for m in range(MT):
                        if m == 0:
                            wg_t, wu_t = wg0, wu0
                        else:
                            wg_t = sb_w.tile([128, KT, 128], bf16, tag="wg",
                                             name="wg_t")
                            nc.sync.dma_start(out=wg_t, in_=Wg[e, m])
                            wu_t = sb_w.tile([128, KT, 128], bf16, tag="wu",
                                             name="wu_t")
                            nc.sync.dma_start(out=wu_t, in_=Wu[e, m])
                        # Preload Wd[m] now; phase B consumes it from SBUF.
                        wd_t = sb_wd.tile([128, D], bf16, tag=f"wd{m}",
                                          name=f"wd{m}")
                        nc.sync.dma_start(out=wd_t, in_=Wd[e, m])
                        wd_tiles.append(wd_t)
                        hg = [psum.tile([128, TB], f32, tag=f"p{t}", name=f"hg{t}")
                              for t in range(NTG)]
                        hu = [psum.tile([128, TB], f32, tag=f"q{t}", name=f"hu{t}")
                              for t in range(NTG)]
                        for wt, ps in ((wg_t, hg), (wu_t, hu)):
                            for k in range(KT):
                                lhsT = wt[:, k, :]
                                for t in range(NTG):
                                    nc.tensor.matmul(
                                        ps[t], lhsT,
                                        xt[k][:, t * TB:(t + 1) * TB],
                                        start=(k == 0), stop=(k == KT - 1))
                        for t in range(NTG):
                            sl = sb_act.tile([128, TB], f32, tag="silu", name="sl")
                            nc.scalar.activation(sl, hg[t], Act.Silu)
                            nc.vector.tensor_mul(
                                ht[:, m * C + t * TB: m * C + (t + 1) * TB],
                                sl, hu[t])

                # Next expert's preamble DMAs go ahead of phase B's
                # Y-store DMAs in the sync queue; their SBUF slots are
                # already free once phase A stops reading them.
                if e + 1 < E_PER_CORE:
                    pre = emit_preamble(e + 1)

                # ---- Phase B: Y[tg] = sum_m H^T[m, tg]^T @ Wd[m] ----
                if True:
                    for tg in range(NTG):
                        tagn = ["p", "q"]
                        ys = [[psum.tile([128, CW], f32,
                                       tag=f"{tagn[n % len(tagn)]}{mt}",
                                       name=f"ys{mt}_{n}")
                               for n in range(NCH)] for mt in range(NMT)]
                        for m in range(MT):
                            wd_t = wd_tiles[m]
                            for mt in range(NMT):
                                col = m * C + tg * TB + mt * 128
                                lhsT = ht[:, col:col + 128]
                                for n in range(NCH):
                                    nc.tensor.matmul(
                                        ys[mt][n], lhsT,
                                        wd_t[:, n * CW:(n + 1) * CW],
                                        start=(m == 0), stop=(m == MT - 1))
                        for mt in range(NMT):
                            scol = tg * NMT + mt
                            for n in range(NCH):
                                yo = sb_out.tile([128, CW], f32, tag="yo",
                                                 name="yo")
                                nc.scalar.activation(
                                    yo, ys[mt][n], Act.Copy,
                                    scale=wv_sb[:, scol:scol + 1])
                                row = tg * TB + mt * 128
                                nc.sync.dma_start(
                                    out=Y[e, row:row + 128,
                                          n * CW:(n + 1) * CW],
                                    in_=yo)
    nc.compile()
    return nc


def _get_nc(C):
    if C not in _BUILD_CACHE:
        _BUILD_CACHE[C] = _build(C)
    return _BUILD_CACHE[C]


def _router_host(x_flat, gate_w):
    """Replicate the reference router + aux loss on JAX-CPU."""
    import jax
    import jax.numpy as jnp

    cpu = jax.devices("cpu")[0]
    with jax.default_device(cpu):
        xj = jnp.asarray(x_flat)
        gj = jnp.asarray(gate_w)
        logits = xj @ gj
        probs = jax.nn.softmax(logits, axis=-1)
        topk_w, topk_idx = jax.lax.top_k(probs, TOP_K)
        topk_wn = topk_w / jnp.sum(topk_w, axis=-1, keepdims=True)
        # aux loss: tokens_per_expert . router_prob_per_expert
        onehot = jax.nn.one_hot(topk_idx, NUM_EXPERTS, dtype=xj.dtype)
        expert_mask = onehot.sum(axis=1)              # [T, E] (K summed)
        tokens_per_expert = expert_mask.mean(axis=0)
        router_prob_per_expert = probs.mean(axis=0)
        aux = AUX_LOSS_COEF * jnp.sum(
            tokens_per_expert * router_prob_per_expert) * NUM_EXPERTS
    return (np.asarray(topk_wn), np.asarray(topk_idx),
            np.asarray(aux, dtype=np.float32))


def kernel(x, gate_w, Wg, Wu, Wd):
    from concourse.bass_utils import run_bass_kernel_spmd

    x = np.asarray(x, dtype=np.float32)
    gate_w = np.asarray(gate_w, dtype=np.float32)
    Wg = np.asarray(Wg, dtype=np.float32)
    Wu = np.asarray(Wu, dtype=np.float32)
    Wd = np.asarray(Wd, dtype=np.float32)

    B, S, _ = x.shape
    T = B * S
    x_flat = x.reshape(T, D)

    topk_wn, topk_idx, aux = _router_host(x_flat, gate_w)

    # Group (token, weight) pairs by expert.
    flat_e = topk_idx.reshape(-1).astype(np.int64)
    flat_w = topk_wn.reshape(-1).astype(np.float32)
    flat_t = np.repeat(np.arange(T, dtype=np.int64), TOP_K)
    order = np.argsort(flat_e, kind="stable")
    sorted_e = flat_e[order]
    tok_by_e = flat_t[order]
    w_by_e = flat_w[order]
    starts = np.searchsorted(sorted_e, np.arange(NUM_EXPERTS + 1))
    counts = np.diff(starts)

    C = max(TB * 3, int(-(-counts.max() // TB)) * TB)
    nc = _get_nc(C)

    x_bf = x_flat.astype(BF16)
    Wg_bf = Wg.astype(BF16)
    Wu_bf = Wu.astype(BF16)
    Wd_bf = Wd.astype(BF16)

    in_maps = []
    tok_lists = []
    for c in range(N_CORES):
        xT_c = np.zeros((E_PER_CORE, KT, 128, C), dtype=BF16)
        wv_c = np.zeros((E_PER_CORE, C // 128, 128), dtype=np.float32)
        Wg_c = np.empty((E_PER_CORE, MT, 128, KT, 128), dtype=BF16)
        Wu_c = np.empty_like(Wg_c)
        Wd_c = np.empty((E_PER_CORE, MT, 128, D), dtype=BF16)
        toks = []
        for j in range(E_PER_CORE):
            e = c * E_PER_CORE + j
            idx = tok_by_e[starts[e]:starts[e + 1]]
            w = w_by_e[starts[e]:starts[e + 1]]
            cnt = len(idx)
            toks.append(idx)
            # [D, cnt] -> [KT, 128, cnt]
            xg = x_bf[idx].T.reshape(KT, 128, cnt)
            xT_c[j, :, :, :cnt] = xg
            wv_c[j].reshape(-1)[:cnt] = w
            # Wg[e]: [D, I] -> [m, p(D in k-tile), k, c(I col)]
            Wg_c[j] = Wg_bf[e].reshape(KT, 128, MT, 128).transpose(2, 1, 0, 3)
            Wu_c[j] = Wu_bf[e].reshape(KT, 128, MT, 128).transpose(2, 1, 0, 3)
            Wd_c[j] = Wd_bf[e].reshape(MT, 128, D)
        tok_lists.append(toks)
        in_maps.append({"xT": xT_c, "Wg": Wg_c, "Wu": Wu_c, "Wd": Wd_c,
                        "wv": wv_c})

    res = run_bass_kernel_spmd(nc, in_maps, core_ids=list(range(N_CORES)))

    out_flat = np.zeros((T, D), dtype=np.float32)
    for c in range(N_CORES):
        Yc = res.results[c]["Y"]
        for j in range(E_PER_CORE):
            idx = tok_lists[c][j]
            out_flat[idx] += Yc[j, :len(idx)]
    return out_flat.reshape(B, S, D), aux
